# revision 14
# baseline (speedup 1.0000x reference)
"""Trainium2 Bass kernel for nn_ActQuantWrapper (per-token 4-bit fake-quant + Linear).

Strategy (8 NeuronCores, SPMD, no collectives):
  - Shard x along the sequence axis: 1024 tokens per core; weight/bias replicated.
  - Host prep: W^T (+bias row appended) cast to bf16; q/fp index masks as uint8 rows.
  - Per core:
      * per-token stats: masked x via memset+copy_predicated, then min/max reduces
        clamped to 0 (exactly reference's min(.,0)/max(.,0) semantics)
      * fake-quant in [token, feature] layout with per-partition scalars; RNE
        round via the +/-1.5*2^23 magic constant fused into ACT/tensor_scalar ops
      * fp-feature override via copy_predicated, cast to bf16
      * DMA-xbar transpose (dma_start_transpose) into [feature, token] tiles
      * bf16 matmul: stationary = mixed^T tile (128x128), streaming = W^T chunk
        (N=512), PSUM accum over 32 feature tiles; bias via a K=1 ones-row matmul;
        ACT drains PSUM -> SBUF -> DMA out.
  - Tokens processed in 2 groups of 512 so W^T streams from HBM twice (96 MB/core
    total traffic) while quant of group g+1 overlaps matmul of group g.
"""

import sys
import numpy as np
import ml_dtypes

sys.path.insert(0, "/opt/trn_rl_repo")

import concourse.bass as bass  # noqa: E402
import concourse.mybir as mybir  # noqa: E402
import concourse.tile as tile  # noqa: E402
from concourse import bacc  # noqa: E402

F32 = mybir.dt.float32
BF16 = mybir.dt.bfloat16
U8 = mybir.dt.uint8

N_CORES = 8
S_FULL, D, O = 8192, 4096, 4096
T = S_FULL // N_CORES          # tokens per core
MAGIC = 12582912.0             # 1.5 * 2**23 : RNE round-to-int for |v| < 2**22
MAXQ = 15.0

N_TT = T // 128                # token tiles per core
GROUPS = 2
TT_PER_G = N_TT // GROUPS      # token tiles per group
G_TOK = T // GROUPS            # tokens per group
CHUNK = 512                    # output-feature chunk per W^T stream tile
N_CH = O // CHUNK
N_DT = D // 128                # feature (contraction) tiles
MT_BUFS = 6

_CACHE = {}


def _build_bass(mode="full"):
    nc = bacc.Bacc("TRN2", target_bir_lowering=False, debug=False,
                   enable_asserts=True, num_devices=N_CORES)
    x_ap = nc.dram_tensor("x", [T, D], F32, kind="ExternalInput").ap()
    wt_ap = nc.dram_tensor("wt", [D + 1, O], BF16, kind="ExternalInput").ap()
    qm_ap = nc.dram_tensor("qmask", [1, D], U8, kind="ExternalInput").ap()
    fm_ap = nc.dram_tensor("fpmask", [1, D], U8, kind="ExternalInput").ap()
    out_ap = nc.dram_tensor("out", [T, O], F32, kind="ExternalOutput").ap()

    with tile.TileContext(nc) as tc:
        _kernel_body(tc, out_ap, x_ap, wt_ap, qm_ap, fm_ap, mode)
    nc.compile()
    return nc


def _kernel_body(tc, out_ap, x_ap, wt_ap, qm_ap, fm_ap, mode="full"):
    from contextlib import ExitStack
    nc = tc.nc
    A = mybir.AluOpType

    with ExitStack() as ctx:
        singles = ctx.enter_context(tc.tile_pool(name="singles", bufs=1))
        xp = ctx.enter_context(tc.tile_pool(name="xp", bufs=2))
        xmp = ctx.enter_context(tc.tile_pool(name="xmp", bufs=2))
        dqp = ctx.enter_context(tc.tile_pool(name="dqp", bufs=2))
        pp = ctx.enter_context(tc.tile_pool(name="pp", bufs=2))
        mtp = ctx.enter_context(tc.tile_pool(name="mtp", bufs=MT_BUFS))
        wcp = ctx.enter_context(tc.tile_pool(name="wcp", bufs=2))
        wbp = ctx.enter_context(tc.tile_pool(name="wbp", bufs=2))
        osp = ctx.enter_context(tc.tile_pool(name="osp", bufs=2))
        pmm = ctx.enter_context(tc.tile_pool(name="pmm", bufs=4, space="PSUM"))

        # --- constants (broadcast index masks across all 128 partitions) ---
        qmask_b = singles.tile([128, D], U8)
        nc.gpsimd.dma_start(out=qmask_b, in_=bass.AP(
            tensor=qm_ap.tensor, offset=qm_ap.offset, ap=[[0, 128], qm_ap.ap[1]]))
        fpmask_b = singles.tile([128, D], U8)
        nc.gpsimd.dma_start(out=fpmask_b, in_=bass.AP(
            tensor=fm_ap.tensor, offset=fm_ap.offset, ap=[[0, 128], fm_ap.ap[1]]))
        ones = singles.tile([1, 128], BF16)
        nc.vector.memset(ones, 1.0)

        for g in range(GROUPS):
            mts = []
            for tt in range(TT_PER_G):
                row = g * G_TOK + tt * 128
                xt = xp.tile([128, D], F32, tag="x")
                nc.sync.dma_start(out=xt, in_=x_ap[row:row + 128, :])

                # xm = x where q-feature else 0  (masked copy)
                xm = xmp.tile([128, D], F32, tag="xm")
                nc.vector.memset(xm, 0.0)
                nc.vector.copy_predicated(xm, qmask_b, xt)

                # rmax = max(max_d xm, 0) ; rmin = min(min_d xm, 0)
                rmax = pp.tile([128, 1], F32, tag="rmax")
                rmin = pp.tile([128, 1], F32, tag="rmin")
                nc.vector.tensor_reduce(rmax, xm, axis=mybir.AxisListType.X, op=A.max)
                nc.vector.tensor_reduce(rmin, xm, axis=mybir.AxisListType.X, op=A.min)
                nc.vector.tensor_scalar(rmax, rmax, 0.0, None, A.max)
                nc.vector.tensor_scalar(rmin, rmin, 0.0, None, A.min)

                # per-token quant params (tiny [128,1] columns)
                rng = pp.tile([128, 1], F32, tag="rng")
                nc.vector.tensor_tensor(rng, rmax, rmin, A.subtract)
                iz2 = pp.tile([128, 1], F32, tag="iz2")       # 2 if range==0 else 0
                nc.vector.tensor_scalar(iz2, rng, 0.0, 2.0, A.is_equal, A.mult)
                rng2 = pp.tile([128, 1], F32, tag="rng2")     # degenerate -> [-1, 1]
                nc.vector.tensor_tensor(rng2, rng, iz2, A.add)
                iz1 = pp.tile([128, 1], F32, tag="iz1")
                nc.vector.tensor_scalar(iz1, iz2, 0.5, None, A.mult)
                xmn = pp.tile([128, 1], F32, tag="xmn")       # xmin' = rmin - is0
                nc.vector.tensor_tensor(xmn, rmin, iz1, A.subtract)
                s = pp.tile([128, 1], F32, tag="s")           # scale = range/15
                nc.vector.tensor_scalar(s, rng2, 1.0 / MAXQ, None, A.mult)
                inv = pp.tile([128, 1], F32, tag="inv")
                nc.vector.reciprocal(inv, s)
                lop = pp.tile([128, 1], F32, tag="lop")       # lo = round(xmin'/scale) = -zero
                nc.scalar.activation(lop, xmn, mybir.ActivationFunctionType.Copy,
                                     bias=MAGIC, scale=inv)
                lo = pp.tile([128, 1], F32, tag="lo")
                nc.vector.tensor_scalar(lo, lop, MAGIC, None, A.subtract)
                hi = pp.tile([128, 1], F32, tag="hi")
                nc.vector.tensor_scalar(hi, lo, MAXQ, None, A.add)

                # quantize (in place on xm):
                # xm <- xm/scale + MAGIC ; xm <- min(xm-MAGIC, hi) ; xm <- max(xm, lo)*scale
                nc.scalar.activation(xm, xm, mybir.ActivationFunctionType.Copy,
                                     bias=MAGIC, scale=inv)
                nc.vector.tensor_scalar(xm, xm, MAGIC, hi, A.subtract, A.min)
                nc.vector.tensor_scalar(xm, xm, lo, s, A.max, A.mult)
                # fp features pass through unquantized
                nc.vector.copy_predicated(xm, fpmask_b, xt)

                if mode == "quant":
                    nc.sync.dma_start(out=out_ap[row:row + 128, :], in_=xm)
                    continue

                # cast to bf16, then DMA-xbar block-transpose:
                # mt[p, j, t] = dq16[t, 128*j + p]
                dq16 = dqp.tile([128, D], BF16, tag="dq16")
                nc.vector.tensor_copy(dq16, xm)
                mt = mtp.tile([128, N_DT, 128], BF16, tag="mt")
                mts.append(mt)
                nc.sync.dma_start_transpose(mt, dq16)

            if mode != "full":
                continue
            # matmul phase for this group: stream W^T chunks, accumulate over feature tiles
            for ch in range(N_CH):
                col = ch * CHUNK
                wtc = wcp.tile([128, N_DT, CHUNK], BF16, tag="wtc")
                nc.sync.dma_start(
                    out=wtc,
                    in_=wt_ap[0:D, col:col + CHUNK].rearrange("(j p) c -> p j c", p=128))
                wbias = wbp.tile([1, CHUNK], BF16, tag="wb")
                nc.sync.dma_start(out=wbias, in_=wt_ap[D:D + 1, col:col + CHUNK])

                for tt in range(TT_PER_G):
                    row = g * G_TOK + tt * 128
                    ps = pmm.tile([128, CHUNK], F32, tag="mm")
                    nc.tensor.matmul(ps, lhsT=ones, rhs=wbias, start=True, stop=False)
                    for j in range(N_DT):
                        nc.tensor.matmul(ps, lhsT=mts[tt][:, j, :], rhs=wtc[:, j, :],
                                         start=False, stop=(j == N_DT - 1))
                    ost = osp.tile([128, CHUNK], F32, tag="ost")
                    nc.scalar.copy(out=ost, in_=ps)
                    nc.sync.dma_start(out=out_ap[row:row + 128, col:col + CHUNK], in_=ost)


def _get_nc():
    if "nc" not in _CACHE:
        _CACHE["nc"] = _build_bass()
    return _CACHE["nc"]


def _prep_in_maps(x, weight, bias, q_idx, fp_idx):
    x = np.ascontiguousarray(np.asarray(x, dtype=np.float32)).reshape(S_FULL, D)
    weight = np.asarray(weight, dtype=np.float32)
    bias = np.asarray(bias, dtype=np.float32)
    q_idx = np.asarray(q_idx).astype(np.int64)
    fp_idx = np.asarray(fp_idx).astype(np.int64)

    wt = np.empty((D + 1, O), dtype=ml_dtypes.bfloat16)
    wt[:D] = weight.T.astype(ml_dtypes.bfloat16)
    wt[D] = bias.astype(ml_dtypes.bfloat16)

    qmask = np.zeros((1, D), dtype=np.uint8)
    qmask[0, q_idx] = 1
    fpmask = np.zeros((1, D), dtype=np.uint8)
    fpmask[0, fp_idx] = 1

    shared = {"wt": wt, "qmask": qmask, "fpmask": fpmask}
    return [
        {"x": np.ascontiguousarray(x[c * T:(c + 1) * T]), **shared}
        for c in range(N_CORES)
    ]


def kernel(x, weight, bias, q_idx, fp_idx):
    from concourse import bass_utils
    bass_utils.upload_artifacts = lambda tmpdir: "local://none"

    nc = _get_nc()
    in_maps = _prep_in_maps(x, weight, bias, q_idx, fp_idx)
    res = bass_utils.run_bass_kernel_spmd(
        nc, in_maps, core_ids=list(range(N_CORES)))
    out = np.concatenate([res.results[c]["out"] for c in range(N_CORES)], axis=0)
    return out.reshape(1, S_FULL, O)
